# Initial kernel scaffold
#
"""Trainium2 Bass kernel for nn_BatchMatMulModule.

Computes out = einsum("bnij,bmj->bnmi", x, y) with
  x: [4, 64, 3, 3] f32, y: [4, 100000, 3] f32 -> out: [4, 64, 100000, 3] f32.

The output (307 MB) dwarfs the inputs (4.8 MB), so the kernel is bound by
HBM write bandwidth (~358 GB/s per NeuronCore). Strategy:

- Shard the 256 flat (b, n) pairs across 8 cores: core c handles b = c // 2
  and n in [32 * (c % 2), 32 * (c % 2) + 32). Each core's output slice
  [32, 100000, 3] is a contiguous chunk of the full output.
- Per core, SBUF layout packs partition dim = (n_sub in 0..4, m_segment in
  0..32) and free dim = (3125 rows x 3 output cols). Each partition's free
  segment is a 9375-float contiguous run of the output, so store-DMAs move
  4.8 MB with 37.5 KB contiguous per partition (near peak DMA efficiency).
- Compute is elementwise: out[:, t, i] = sum_j y[:, t, j] * x[n, i, j].
  y is resident in SBUF (replicated over the 4 n_sub partition groups);
  x values are per-partition scalars. The j = 0 term runs on the Scalar
  engine (activation Copy with per-partition scale); the j = 1, 2 terms are
  fused multiply-accumulates (scalar_tensor_tensor) on Vector/GPSIMD.
"""

import numpy as np

import concourse.bass as bass
import concourse.mybir as mybir
from concourse.bass_utils import run_bass_kernel_spmd
from concourse.tile import TileContext

N_CORES = 8
P = 128
N_PER_CORE = 32  # (b, n) pairs per core
N_SUB = 4        # n values packed across partition groups
N_GROUPS = N_PER_CORE // N_SUB  # 8 groups, one out tile each
SEGS = P // N_SUB               # 32 m-segments per n
M = 100000
ROWS = M // SEGS                # 3125 rows per partition
FREE = ROWS * 3                 # 9375 f32 per partition

TRACE = False
LAST = None  # last BassKernelResults, for test harness introspection

_CACHED_NC = None


def build_bass():
    nc = bass.Bass(
        "TRN2",
        debug=False,
        enable_asserts=False,
        target_bir_lowering=False,
        num_devices=N_CORES,
    )
    f32 = mybir.dt.float32
    xs = nc.dram_tensor("xs", [N_PER_CORE, 3, 3], f32, kind="ExternalInput").ap()
    ys = nc.dram_tensor("ys", [M, 3], f32, kind="ExternalInput").ap()
    out = nc.dram_tensor("out", [N_PER_CORE, M, 3], f32, kind="ExternalOutput").ap()

    mult = mybir.AluOpType.mult
    add = mybir.AluOpType.add
    copy = mybir.ActivationFunctionType.Copy

    with TileContext(nc) as tc:
        with (
            tc.tile_pool(name="const", bufs=1) as cpool,
            tc.tile_pool(name="outp", bufs=3) as opool,
        ):
            # y resident in SBUF: partition (a, s) holds y rows
            # [s*ROWS, (s+1)*ROWS) — identical for every a group.
            y_tile = cpool.tile([P, FREE], f32)
            y_src = ys.rearrange("(s t) i -> s (t i)", s=SEGS)  # [32, 9375]
            for a in range(N_SUB):
                nc.sync.dma_start(out=y_tile[a * SEGS:(a + 1) * SEGS, :], in_=y_src)

            # x scalars: partition (a, s) holds x[g*4 + a, i, j] at free
            # index g*9 + i*3 + j.
            xsb = cpool.tile([P, N_GROUPS * 9], f32)
            xs_rows = xs.rearrange("(g a) i j -> a (g i j)", a=N_SUB)  # [4, 72]
            for a in range(N_SUB):
                nc.sync.dma_start(
                    out=xsb[a * SEGS:(a + 1) * SEGS, :],
                    in_=xs_rows[a:a + 1, :].to_broadcast((SEGS, N_GROUPS * 9)),
                )

            yv = y_tile.rearrange("p (t i) -> p t i", i=3)
            for g in range(N_GROUPS):
                ot = opool.tile([P, FREE], f32, name=f"ot{g}", tag="ot")
                ov = ot.rearrange("p (t i) -> p t i", i=3)
                for i in range(3):
                    c = g * 9 + i * 3
                    nc.scalar.activation(
                        out=ov[:, :, i], in_=yv[:, :, 0], func=copy,
                        scale=xsb[:, c:c + 1],
                    )
                    nc.vector.scalar_tensor_tensor(
                        out=ov[:, :, i], in0=yv[:, :, 1],
                        scalar=xsb[:, c + 1:c + 2], in1=ov[:, :, i],
                        op0=mult, op1=add,
                    )
                    nc.vector.scalar_tensor_tensor(
                        out=ov[:, :, i], in0=yv[:, :, 2],
                        scalar=xsb[:, c + 2:c + 3], in1=ov[:, :, i],
                        op0=mult, op1=add,
                    )
                # Store: partition (a, s) -> out[g*4 + a, s*ROWS:(s+1)*ROWS, :]
                dst = out[g * N_SUB:(g + 1) * N_SUB].rearrange(
                    "a (s t) i -> (a s) (t i)", s=SEGS
                )
                nc.sync.dma_start(out=dst, in_=ot[:])
    return nc


def kernel(x: np.ndarray, y: np.ndarray) -> np.ndarray:
    global LAST, _CACHED_NC
    x = np.ascontiguousarray(x, dtype=np.float32)
    y = np.ascontiguousarray(y, dtype=np.float32)
    assert x.shape == (4, 64, 3, 3) and y.shape == (4, 100000, 3)

    if _CACHED_NC is None:
        _CACHED_NC = build_bass()
    nc = _CACHED_NC

    x_flat = x.reshape(256, 3, 3)
    in_maps = []
    for c in range(N_CORES):
        b = c // 2
        in_maps.append({
            "xs": x_flat[c * N_PER_CORE:(c + 1) * N_PER_CORE],
            "ys": y[b],
        })

    res = run_bass_kernel_spmd(
        nc, in_maps, core_ids=list(range(N_CORES)), trace=TRACE,
    )
    LAST = res
    out = np.concatenate([r["out"] for r in res.results], axis=0)
    return out.reshape(4, 64, 100000, 3)


# revision 16
# speedup vs baseline: 1.0613x; 1.0613x over previous
"""Trainium2 Bass kernel for nn_BatchMatMulModule.

Computes out = einsum("bnij,bmj->bnmi", x, y) with
  x: [4, 64, 3, 3] f32, y: [4, 100000, 3] f32 -> out: [4, 64, 100000, 3] f32.

The output (307 MB) dwarfs the inputs (4.8 MB), so the kernel is bound by
HBM write bandwidth (~358 GB/s per NeuronCore). Strategy:

- Shard the 256 flat (b, n) pairs across 8 cores: core c handles b = c // 2
  and n in [32 * (c % 2), 32 * (c % 2) + 32). Each core's output slice
  [32, 100000, 3] is a contiguous chunk of the full output.
- Per core, SBUF layout packs partition dim = (n_sub in 0..4, m_segment in
  0..32) and free dim = (3125 rows x 3 output cols). Each partition's free
  segment is a 9375-float contiguous run of the output, so store-DMAs move
  4.8 MB with 37.5 KB contiguous per partition (near peak DMA efficiency).
- Compute is elementwise: out[:, t, i] = sum_j y[:, t, j] * x[n, i, j].
  y is resident in SBUF (replicated over the 4 n_sub partition groups);
  x values are per-partition scalars. The j = 0 term runs on the Scalar
  engine (activation Copy with per-partition scale); the j = 1, 2 terms are
  fused multiply-accumulates (scalar_tensor_tensor) on Vector/GPSIMD.
"""

import numpy as np

import concourse.bacc as bacc
import concourse.mybir as mybir
from concourse.bass_utils import run_bass_kernel_spmd
from concourse.tile import TileContext

N_CORES = 8
P = 128
N_PER_CORE = 32  # (b, n) pairs per core
N_SUB = 4        # n values packed across partition groups
N_GROUPS = N_PER_CORE // N_SUB  # 8 groups, one out tile each
SEGS = P // N_SUB               # 32 m-segments per n
M = 100000
ROWS = M // SEGS                # 3125 rows per partition
FREE = ROWS * 3                 # 9375 f32 per partition

TRACE = False
LAST = None  # last BassKernelResults, for test harness introspection

_CACHED_NC = None


def build_bass(reps: int = 1):
    nc = bacc.Bacc(
        "TRN2",
        debug=False,
        enable_asserts=False,
        target_bir_lowering=False,
        num_devices=N_CORES,
    )
    f32 = mybir.dt.float32
    # xs arrives host-pre-expanded to the per-partition scalar layout:
    # xs[p = a*32 + s, col = g*9 + i*3 + j] = x[g*4 + a, i, j].
    xs = nc.dram_tensor("xs", [P, N_GROUPS * 9], f32, kind="ExternalInput").ap()
    ys = nc.dram_tensor("ys", [M, 3], f32, kind="ExternalInput").ap()
    out = nc.dram_tensor("out", [N_PER_CORE, M, 3], f32, kind="ExternalOutput").ap()

    mult = mybir.AluOpType.mult
    add = mybir.AluOpType.add
    copy = mybir.ActivationFunctionType.Copy

    with TileContext(nc) as tc:
        with (
            tc.tile_pool(name="const", bufs=1) as cpool,
            tc.tile_pool(name="outp", bufs=3) as opool,
        ):
            # y resident in SBUF: partition (a, s) holds y rows
            # [s*ROWS, (s+1)*ROWS) — identical for every a group.
            y_tile = cpool.tile([P, FREE], f32)
            y_src = ys.rearrange("(s t) i -> s (t i)", s=SEGS)  # [32, 9375]
            nc.sync.dma_start(
                out=y_tile[:],
                in_=y_src.unsqueeze(0).to_broadcast((N_SUB, SEGS, FREE)),
            )

            # x scalars: partition (a, s) holds x[g*4 + a, i, j] at free
            # index g*9 + i*3 + j.
            xsb = cpool.tile([P, N_GROUPS * 9], f32)
            nc.sync.dma_start(out=xsb[:], in_=xs)

            # Single sync point: compute waits on this barrier instead of
            # on every setup DMA individually (ISA wait-slot limit).
            tc.strict_bb_all_engine_barrier()

            yv = y_tile.rearrange("p (t i) -> p t i", i=3)
            for g in range(N_GROUPS * reps):
                g = g % N_GROUPS
                ot = opool.tile([P, FREE], f32, name=f"ot{g}", tag="ot")
                ov = ot.rearrange("p (t i) -> p t i", i=3)
                for i in range(3):
                    c = g * 9 + i * 3
                    nc.scalar.activation(
                        out=ov[:, :, i], in_=yv[:, :, 0], func=copy,
                        scale=xsb[:, c:c + 1],
                    )
                    nc.vector.scalar_tensor_tensor(
                        out=ov[:, :, i], in0=yv[:, :, 1],
                        scalar=xsb[:, c + 1:c + 2], in1=ov[:, :, i],
                        op0=mult, op1=add,
                    )
                    nc.vector.scalar_tensor_tensor(
                        out=ov[:, :, i], in0=yv[:, :, 2],
                        scalar=xsb[:, c + 2:c + 3], in1=ov[:, :, i],
                        op0=mult, op1=add,
                    )
                # Store: partition (a, s) -> out[g*4 + a, s*ROWS:(s+1)*ROWS, :]
                dst = out[g * N_SUB:(g + 1) * N_SUB].rearrange(
                    "a (s t) i -> (a s) (t i)", s=SEGS
                )
                nc.sync.dma_start(out=dst, in_=ot[:])
    nc.compile()
    return nc


def kernel(x: np.ndarray, y: np.ndarray) -> np.ndarray:
    global LAST, _CACHED_NC
    x = np.ascontiguousarray(x, dtype=np.float32)
    y = np.ascontiguousarray(y, dtype=np.float32)
    assert x.shape == (4, 64, 3, 3) and y.shape == (4, 100000, 3)

    if _CACHED_NC is None:
        _CACHED_NC = build_bass()
    nc = _CACHED_NC

    x_flat = x.reshape(256, 3, 3)
    in_maps = []
    for c in range(N_CORES):
        b = c // 2
        xl = x_flat[c * N_PER_CORE:(c + 1) * N_PER_CORE]  # [32, 3, 3]
        per_a = xl.reshape(N_GROUPS, N_SUB, 9).transpose(1, 0, 2).reshape(N_SUB, 72)
        xsb_np = np.ascontiguousarray(np.repeat(per_a, SEGS, axis=0))  # [128, 72]
        in_maps.append({"xs": xsb_np, "ys": y[b]})

    res = run_bass_kernel_spmd(
        nc, in_maps, core_ids=list(range(N_CORES)), trace=TRACE,
    )
    LAST = res
    out = np.concatenate([r["out"] for r in res.results], axis=0)
    return out.reshape(4, 64, 100000, 3)


def _make_in_maps(x, y):
    x_flat = x.reshape(256, 3, 3)
    in_maps = []
    for c in range(N_CORES):
        b = c // 2
        xl = x_flat[c * N_PER_CORE:(c + 1) * N_PER_CORE]
        per_a = xl.reshape(N_GROUPS, N_SUB, 9).transpose(1, 0, 2).reshape(N_SUB, 72)
        xsb_np = np.ascontiguousarray(np.repeat(per_a, SEGS, axis=0))
        in_maps.append({"xs": xsb_np, "ys": y[b]})
    return in_maps


def _prepare_exec(nc, in_maps):
    """Build a jitted 8-core executor for `nc` with device-resident inputs.

    Returns (run_once, ins_dev, zeros) where run_once(outs) executes the
    NEFF once per core and returns new device outputs (pass them back in as
    the donated output buffers for the next call)."""
    import jax
    import concourse.mybir as mybir_
    from jax.experimental.shard_map import shard_map
    from jax.sharding import Mesh, NamedSharding, PartitionSpec
    from concourse.bass2jax import (
        _bass_exec_p, install_neuronx_cc_hook, partition_id_tensor,
    )

    install_neuronx_cc_hook()
    partition_name = nc.partition_id_tensor.name if nc.partition_id_tensor else None
    in_names, out_names, out_avals, zero_outs = [], [], [], []
    for alloc in nc.m.functions[0].allocations:
        if not isinstance(alloc, mybir_.MemoryLocationSet):
            continue
        name = alloc.memorylocations[0].name
        if alloc.kind == "ExternalInput":
            if name != partition_name:
                in_names.append(name)
        elif alloc.kind == "ExternalOutput":
            shape = tuple(alloc.tensor_shape)
            dtype = mybir_.dt.np(alloc.dtype)
            out_names.append(name)
            out_avals.append(jax.core.ShapedArray(shape, dtype))
            zero_outs.append(np.zeros(shape, dtype))
    n_params = len(in_names)
    n_outs = len(out_names)
    all_names = in_names + out_names + ([partition_name] if partition_name else [])

    def _body(*args):
        operands = list(args)
        if partition_name is not None:
            operands.append(partition_id_tensor())
        outs = _bass_exec_p.bind(
            *operands,
            out_avals=tuple(out_avals),
            in_names=tuple(all_names),
            out_names=tuple(out_names),
            lowering_input_output_aliases=(),
            sim_require_finite=True,
            sim_require_nnan=True,
            nc=nc,
        )
        return tuple(outs)

    devices = jax.devices()[:N_CORES]
    mesh = Mesh(np.asarray(devices), ("core",))
    spec = PartitionSpec("core")
    sharded = jax.jit(
        shard_map(
            _body, mesh=mesh, in_specs=(spec,) * (n_params + n_outs),
            out_specs=(spec,) * n_outs, check_rep=False,
        ),
        donate_argnums=tuple(range(n_params, n_params + n_outs)),
        keep_unused=True,
    )
    sh = NamedSharding(mesh, spec)
    ins_dev = [
        jax.device_put(
            np.concatenate([np.asarray(m[name]) for m in in_maps], axis=0), sh
        )
        for name in in_names
    ]
    zeros = [
        jax.device_put(
            np.zeros((N_CORES * z.shape[0], *z.shape[1:]), z.dtype), sh
        )
        for z in zero_outs
    ]

    def run_once(outs):
        res = sharded(*ins_dev, *outs)
        jax.block_until_ready(res)
        return list(res)

    return run_once, zeros


def bench(x, y, reps_pair=(1, 33), samples=20):
    """Measure steady-state per-workload HW time by differencing kernels
    that run the workload `reps_pair[0]` vs `reps_pair[1]` times."""
    import time
    x = np.ascontiguousarray(x, dtype=np.float32)
    y = np.ascontiguousarray(y, dtype=np.float32)
    in_maps = _make_in_maps(x, y)
    times = {}
    for reps in reps_pair:
        nc = build_bass(reps=reps)
        run_once, zeros = _prepare_exec(nc, in_maps)
        outs = run_once(zeros)  # compile + warm
        ts = []
        for _ in range(samples):
            t0 = time.perf_counter()
            outs = run_once(outs)
            ts.append(time.perf_counter() - t0)
        ts.sort()
        times[reps] = ts[0]
        print(f"reps={reps}: min call {times[reps]*1e3:.2f} ms  "
              f"p25 {ts[len(ts)//4]*1e3:.2f} ms")
    r1, r2 = reps_pair
    per_iter_s = (times[r2] - times[r1]) / (r2 - r1)
    return per_iter_s * 1e9
